# revision 29
# baseline (speedup 1.0000x reference)
"""Causal multi-head attention on 8 Trainium2 NeuronCores.

Full module: x:(2,2048,1024) f32, 16 heads, head_dim 64, causal softmax
(scaled by 1/sqrt(1024)), out = attn(x) @ Wo + bo.

Sharding: core c handles batch b = c // 4 and head group g = c % 4
(4 heads, i.e. 256 columns of Wq/Wk/Wv and 256 rows of Wo). Every core
runs the same program (SPMD); the host sums the 4 per-group partial
output projections per batch and adds the bias.

Per-core kernel layout strategy (all matmuls contract over the SBUF
partition dim; "T" tensors are stored feature-major so no transposes are
needed mid-attention):
  xT   [f=1024, t=2048]  bf16, built via PE transposes + cast on evict
  QT/KT[d=256,  t=2048]  = Wq/Wk as lhsT, xT as rhs  (2 tiles, head pairs)
  V    [t=2048, d=256]   = xT as lhsT, Wv as rhs; stored [128,16,4,65]
                          with a ones column per head (fused softmax sums)
  S^T  [k=128, q=512]    = KT-slice as lhsT, QT-slice as rhs, two heads
                          run on disjoint partition halves (row-tiled)
  P^T  = exp(S^T/32) via ScalarE, causal mask applied on diagonal blocks
  ctxT [d=64|sums, q]    = [V|1] as lhsT, P^T as rhs, accumulated in PSUM
  norm: recip(sums row) -> PE outer-product broadcast -> DVE multiply
  out  [t, 1024]         = ctxT as lhsT, Wo as rhs  (partial; host-summed)
"""

import os

import numpy as np

N = 2048        # tokens per batch
D = 1024        # model dim
HG = 4          # heads per core
HD = 64         # head dim
DG = HG * HD    # 256, feature columns per core
SCALE = 1.0 / 32.0  # 1/sqrt(D); note module scales by sqrt(d_out), not head_dim
NCORES = 8

# tuning knobs (env-overridable for experiments)
OUTER_F32R = os.environ.get("OUTER_F32R", "1") != "0"  # f32r outer products
PT_BUFS = int(os.environ.get("PT_BUFS", "6"))
HOST_XT = os.environ.get("HOST_XT", "0") != "0"      # host passes x already transposed

_CACHE = {}


def _build_nc(repeat=1):
    from contextlib import ExitStack

    import concourse.mybir as mybir
    import concourse.tile as tile
    from concourse import bacc
    from concourse.masks import make_identity

    FP32 = mybir.dt.float32
    F32R = mybir.dt.float32r
    BF16 = mybir.dt.bfloat16
    EXP = mybir.ActivationFunctionType.Exp
    COPY = mybir.ActivationFunctionType.Copy

    NT = N // 128   # 16 token chunks
    NF = D // 128   # 8 feature chunks
    NQ = N // 512   # 4 query blocks

    nc = bacc.Bacc("TRN2", target_bir_lowering=False, debug=False)

    if HOST_XT:
        x_d = nc.dram_tensor("x", [D, N], BF16, kind="ExternalInput").ap()
    else:
        x_d = nc.dram_tensor("x", [N, D], FP32, kind="ExternalInput").ap()
    wq_d = nc.dram_tensor("wq", [D, DG], FP32, kind="ExternalInput").ap()
    wk_d = nc.dram_tensor("wk", [D, DG], FP32, kind="ExternalInput").ap()
    wv_d = nc.dram_tensor("wv", [D, DG], FP32, kind="ExternalInput").ap()
    wo_d = nc.dram_tensor("wo", [DG, D], FP32, kind="ExternalInput").ap()
    out_d = nc.dram_tensor("out", [N, D], FP32, kind="ExternalOutput").ap()

    with tile.TileContext(nc) as tc, ExitStack() as ctx:
        persist = ctx.enter_context(tc.tile_pool(name="persist", bufs=1))
        xpool = ctx.enter_context(tc.tile_pool(name="xpool", bufs=8))
        wstage = ctx.enter_context(tc.tile_pool(name="wstage", bufs=2))
        ptpool = ctx.enter_context(tc.tile_pool(name="ptpool", bufs=PT_BUFS))
        stpool = ctx.enter_context(tc.tile_pool(name="stpool", bufs=4))
        smpool = ctx.enter_context(tc.tile_pool(name="smpool", bufs=4))
        opool = ctx.enter_context(tc.tile_pool(name="opool", bufs=3))
        # PSUM budget (8 banks): proj/transpose/outproj tag "ps"
        # [128,512]x2 = 2 banks; attention S tag "ps_s" [128,1024]x2 = 4
        # banks (independent rotation domains so the phases pipeline);
        # pv pool 2 banks, slots reused for the broadcast outer-products.
        mmpsum = ctx.enter_context(tc.tile_pool(name="mmpsum", bufs=2, space="PSUM"))
        spsum = mmpsum
        pvpsum = ctx.enter_context(tc.tile_pool(name="pvpsum", bufs=1, space="PSUM"))

        # ---- persistent tensors ----
        xT = persist.tile([128, NF, N], BF16, name="xT")          # 32 KB/p
        qt = persist.tile([128, 2, N], BF16, name="qt")           # 8 KB/p
        kt = persist.tile([128, 2, N], BF16, name="kt")           # 8 KB/p
        vt = persist.tile([128, NT, HG, HD + 1], BF16, name="vt")  # ~8 KB/p
        ctxT = persist.tile([128, 2, N], BF16, name="ctxT")       # 8 KB/p
        wq_bf = persist.tile([128, NF, DG], BF16, name="wq_bf")   # 4 KB/p
        wk_bf = persist.tile([128, NF, DG], BF16, name="wk_bf")
        wv_bf = persist.tile([128, NF, DG], BF16, name="wv_bf")
        wo_bf = persist.tile([128, 2, D], BF16, name="wo_bf")     # 4 KB/p
        ones128 = persist.tile([128, HD], F32R if OUTER_F32R else FP32,
                               name="ones128")
        if not HOST_XT:
            ident = persist.tile([128, 128], FP32, name="ident")
            make_identity(nc, ident[:, :])

        if OUTER_F32R:
            # walrus requires f32r operands produced by a rounding op
            ones_f32 = persist.tile([128, HD], FP32, name="ones_f32")
            nc.gpsimd.memset(ones_f32[:, :], 1.0)
            nc.vector.tensor_copy(ones128[:, :], ones_f32[:, :])
        else:
            nc.gpsimd.memset(ones128[:, :], 1.0)
        nc.gpsimd.memset(vt[:, :, :, HD], 1.0)  # softmax-sum ones columns

        def emit_weights():
            # ---- weights: load + cast to bf16 (casts on ScalarE: idle here) ----
            for w_dram, w_bf in ((wq_d, wq_bf), (wk_d, wk_bf), (wv_d, wv_bf)):
                w_st = wstage.tile([128, NF, DG], FP32, name="w_st")
                nc.sync.dma_start(out=w_st[:, :, :],
                                  in_=w_dram.rearrange("(c p) d -> p c d", p=128))
                nc.scalar.activation(w_bf[:, :, :], w_st[:, :, :], COPY)
            wo_st = wstage.tile([128, 2, D], FP32, name="w_st")
            nc.sync.dma_start(out=wo_st[:, :, :],
                              in_=wo_d.rearrange("(c p) d -> p c d", p=128))
            nc.scalar.activation(wo_bf[:, :, :], wo_st[:, :, :], COPY)

        def emit_xt_block(ib):
            """Build the bf16 xT slab for one 512-token range."""
            if HOST_XT:
                # x arrives feature-major in bf16: straight DMA into xT
                nc.sync.dma_start(
                    out=xT[:, :, 512 * ib:512 * (ib + 1)],
                    in_=x_d.rearrange("(c p) t -> p c t", p=128)[
                        :, :, 512 * ib:512 * (ib + 1)],
                )
                return
            for u in range(4):
                ti = 4 * ib + u
                t0 = ti * 128
                xn_t = xpool.tile([128, D], FP32, name="xn")
                nc.sync.dma_start(out=xn_t[:, :], in_=x_d[t0:t0 + 128, :])
                for jh in range(2):        # f chunks [4jh .. 4jh+3]
                    ps_tr = mmpsum.tile([128, 512], FP32, name="ps",
                                        tag="ps")
                    for v in range(4):
                        j = 4 * jh + v
                        nc.tensor.transpose(
                            ps_tr[:, 128 * v:128 * (v + 1)],
                            xn_t[:, 128 * j:128 * (j + 1)],
                            ident[:, :],
                        )
                    nc.vector.tensor_copy(
                        xT[:, 4 * jh:4 * jh + 4, 128 * ti:128 * (ti + 1)],
                        ps_tr[:, :].rearrange("p (j t) -> p j t", j=4))

        def emit_proj_mms(ib):
            """Project one 512-token range of xT into QT/KT/V."""
            tb = ib
            for w_bf, dst in ((wq_bf, qt), (wk_bf, kt)):
                for dh in range(2):
                    ps = mmpsum.tile([128, 512], FP32, name="ps", tag="ps")
                    for fc in range(NF):
                        nc.tensor.matmul(
                            ps[:, :],
                            lhsT=w_bf[:, fc, 128 * dh:128 * (dh + 1)],
                            rhs=xT[:, fc, 512 * tb:512 * (tb + 1)],
                            start=(fc == 0), stop=(fc == NF - 1),
                        )
                    nc.vector.tensor_copy(
                        dst[:, dh, 512 * tb:512 * (tb + 1)], ps[:, :])
            for tcc in range(4 * ib, 4 * ib + 4):
                ps = mmpsum.tile([128, 512], FP32, name="ps", tag="ps")
                for fc in range(NF):
                    nc.tensor.matmul(
                        ps[:, 0:DG],
                        lhsT=xT[:, fc, 128 * tcc:128 * (tcc + 1)],
                        rhs=wv_bf[:, fc, :],
                        start=(fc == 0), stop=(fc == NF - 1),
                    )
                nc.vector.tensor_copy(
                    vt[:, tcc, :, 0:HD],
                    ps[:, 0:DG].rearrange("p (h e) -> p h e", h=HG))

        def emit_attention(qb):
            """Attention for one 512-wide query block, both head pairs,
            then the output projection for the same token range."""
            nkc = 4 * (qb + 1)             # causal: k chunks 0..4qb+3
            qsl = slice(512 * qb, 512 * (qb + 1))
            for p in range(2):             # head pair (heads 2p, 2p+1)
                # S + exp + PV stream (PV lags one chunk behind exp)
                pv_a = pvpsum.tile([HD + 1, 512], FP32, name="pv_a",
                                   tag="pv_a")
                pv_b = pvpsum.tile([HD + 1, 512], FP32, name="pv_b",
                                   tag="pv_b")
                for kc in range(nkc):
                    ksl = slice(128 * kc, 128 * (kc + 1))
                    # columns q_local < 128*m are entirely above the
                    # causal diagonal for this k chunk: skip them.
                    m = max(0, kc - 4 * qb)
                    q0 = 128 * m
                    ps_s = spsum.tile([128, 1024], FP32, name="ps_s",
                                      tag="ps_s", bufs=2)
                    # head A on partitions 0-63, head B on 64-127
                    for i in range(2):
                        lo = 64 * i
                        nc.tensor.matmul(
                            ps_s[:, 512 * i:512 * (i + 1)],
                            lhsT=kt[lo:lo + 64, p, ksl],
                            rhs=qt[lo:lo + 64, p, qsl],
                            start=True, stop=True,
                        )
                    pt = ptpool.tile([128, 1024], BF16, name="pt")
                    # full-width exp even on narrowed diagonal chunks: the
                    # skipped columns hold stale PSUM (finite, O(1)) and
                    # are never read downstream
                    nc.scalar.activation(pt[:, :], ps_s[:, :], EXP,
                                         scale=SCALE)
                    if kc >= 4 * qb:       # diagonal: zero q < k in
                        # place on the columns PV will actually read
                        for i in range(2):
                            sl = slice(512 * i + q0, 512 * (i + 1))
                            nc.gpsimd.affine_select(
                                out=pt[:, sl], in_=pt[:, sl],
                                compare_op=mybir.AluOpType.is_ge,
                                fill=0.0,
                                base=0,
                                pattern=[[1, 512 - q0]],
                                channel_multiplier=-1,
                            )
                    st = (kc == 0)
                    sp = (kc == nkc - 1)
                    nc.tensor.matmul(
                        pv_a[:, q0:512], lhsT=vt[:, kc, 2 * p, :],
                        rhs=pt[:, q0:512], start=st, stop=sp,
                    )
                    nc.tensor.matmul(
                        pv_b[:, q0:512], lhsT=vt[:, kc, 2 * p + 1, :],
                        rhs=pt[:, 512 + q0:1024], start=st, stop=sp,
                    )
                # epilogue: stage PSUM out (frees pv slots for the bc
                # outer-products), then normalize by the fused sums row
                st_a = stpool.tile([HD + 1, 512], FP32, name="st_a", tag="st")
                st_b = stpool.tile([HD + 1, 512], FP32, name="st_b", tag="st")
                nc.vector.tensor_copy(st_a[:, :], pv_a[:, :])
                nc.vector.tensor_copy(st_b[:, :], pv_b[:, :])
                rec = smpool.tile([HD + 1, 1024],
                                  F32R if OUTER_F32R else FP32, name="rec")
                with nc.allow_low_precision(reason="f32r softmax recip"):
                    nc.vector.reciprocal(rec[HD:HD + 1, 0:512],
                                         st_a[HD:HD + 1, :])
                    nc.vector.reciprocal(rec[HD:HD + 1, 512:1024],
                                         st_b[HD:HD + 1, :])
                bc_a = pvpsum.tile([HD, 512], FP32, name="bc_a", tag="pv_a")
                bc_b = pvpsum.tile([HD, 512], FP32, name="bc_b", tag="pv_b")
                ones_ap = ones128[HD:HD + 1, :]
                rec_a = rec[HD:HD + 1, 0:512]
                rec_b = rec[HD:HD + 1, 512:1024]
                nc.tensor.matmul(bc_a[:, :], lhsT=ones_ap, rhs=rec_a,
                                 start=True, stop=True)
                nc.tensor.matmul(bc_b[:, :], lhsT=ones_ap, rhs=rec_b,
                                 start=True, stop=True)
                # head A lands on ctxT partitions 0-63 directly
                nc.vector.tensor_mul(ctxT[0:HD, p, qsl], st_a[0:HD, :],
                                     bc_a[:, :])
                # head B: multiply at partitions 0-63, DMA to 64-127
                cb = stpool.tile([HD, 512], BF16, name="cb", tag="cb")
                nc.vector.tensor_mul(cb[:, :], st_b[0:HD, :], bc_b[:, :])
                nc.sync.dma_start(out=ctxT[HD:128, p, qsl], in_=cb[:, :])
            # output projection for this token range (partial over heads)
            for tb in range(4 * qb, 4 * qb + 4):
                tsl = slice(128 * tb, 128 * (tb + 1))
                for nh in range(2):
                    ps_o = mmpsum.tile([128, 512], FP32, name="ps", tag="ps")
                    for hc in range(2):
                        nc.tensor.matmul(
                            ps_o[:, :],
                            lhsT=ctxT[:, hc, tsl],
                            rhs=wo_bf[:, hc, 512 * nh:512 * (nh + 1)],
                            start=(hc == 0), stop=(hc == 1),
                        )
                    o_sb = opool.tile([128, 512], FP32, name="o_sb")
                    if nh == 0 or qb == NQ - 1:
                        nc.vector.tensor_copy(o_sb[:, :], ps_o[:, :])
                    else:
                        nc.scalar.activation(o_sb[:, :], ps_o[:, :], COPY)
                    nc.sync.dma_start(
                        out=out_d[tsl, 512 * nh:512 * (nh + 1)],
                        in_=o_sb[:, :])

        def emit_body():
            emit_xt_block(0)
            emit_weights()
            emit_proj_mms(0)
            emit_xt_block(1)
            emit_proj_mms(1)
            emit_attention(0)
            emit_xt_block(2)
            emit_proj_mms(2)
            emit_attention(1)
            emit_xt_block(3)
            emit_proj_mms(3)
            emit_attention(2)
            emit_attention(3)

        for _rep in range(repeat):
            emit_body()

    nc.compile()
    return nc


def _get_nc(repeat=1):
    key = ("nc", repeat)
    if key not in _CACHE:
        _CACHE[key] = _build_nc(repeat)
    return _CACHE[key]


def _make_in_maps(x, Wq, Wk, Wv, Wo):
    in_maps = []
    for c in range(NCORES):
        b, g = divmod(c, 4)
        cs = slice(DG * g, DG * (g + 1))
        if HOST_XT:
            import ml_dtypes
            xb = np.ascontiguousarray(x[b].T).astype(ml_dtypes.bfloat16)
        else:
            xb = np.ascontiguousarray(x[b], dtype=np.float32)
        in_maps.append({
            "x": xb,
            "wq": np.ascontiguousarray(Wq[:, cs], dtype=np.float32),
            "wk": np.ascontiguousarray(Wk[:, cs], dtype=np.float32),
            "wv": np.ascontiguousarray(Wv[:, cs], dtype=np.float32),
            "wo": np.ascontiguousarray(Wo[cs, :], dtype=np.float32),
        })
    return in_maps


def _gather(results, bo):
    out = np.empty((2, N, D), dtype=np.float32)
    for b in range(2):
        acc = results[4 * b]["out"].astype(np.float32)
        for g in range(1, 4):
            acc = acc + results[4 * b + g]["out"]
        out[b] = acc + bo[None, :].astype(np.float32)
    return out


def run_spmd(x, Wq, Wk, Wv, Wo, bo, **spmd_kwargs):
    """Run the 8-core kernel; returns (full_output, BassKernelResults)."""
    from concourse.bass_utils import run_bass_kernel_spmd

    nc = _get_nc()
    in_maps = _make_in_maps(
        np.asarray(x), np.asarray(Wq), np.asarray(Wk), np.asarray(Wv),
        np.asarray(Wo))
    res = run_bass_kernel_spmd(nc, in_maps, core_ids=list(range(NCORES)),
                               **spmd_kwargs)
    return _gather(res.results, np.asarray(bo)), res


def kernel(x, Wq, Wk, Wv, Wo, bo):
    out, _ = run_spmd(x, Wq, Wk, Wv, Wo, bo)
    return out

